# revision 17
# baseline (speedup 1.0000x reference)
"""AtomNet_V surface-point kernel for one TRN2 chip (8 NeuronCores).

Reference computation (see problem statement):
  - 3-layer MLP on atom types -> per-atom chemical features t [M, 16]
  - per surface point: top-16 nearest atoms (by squared distance),
    distance-weighted rank-attention reduction of (vec outer feat),
    L2 over the spatial axis, then a 3-layer output MLP -> [N, 16]

Distribution: surface points (N=16384) sharded 2048/core across 8 cores;
atoms + weights replicated. No cross-core communication.

Per-core pipeline (all compute on device):
  setup:  transposed loads of atoms/types, |a|^2 via ones-matmul,
          atom MLP in [chem, atom] layout on PE, gather table [M, 20]
          (16 feat cols + 3 xyz cols + pad) written to DRAM scratch.
  chunk loop (16 chunks of 128 points):
    - PE matmul with contraction dim 4 computes s = 2*x.a - |a|^2
      (monotone rank key: argmax s == argmin distance)
    - DVE max/max_index/match_replace/max/max_index -> rank-ordered
      top-16 values + indices (exactly matches lax.top_k ordering)
    - 16 indirect DMAs gather the selected atom rows (feat+xyz)
    - w_k = 1/(|x|^2 - s_k + 1e-8); u_k = w_k * W_att[k]
    - fx[c,d] = x_d * sum_k u_k t_kc - sum_k u_k t_kc a_kd ; fx = |fx|_d
    - PE transpose + 3 16x16 matmuls (leaky relu) -> out rows
"""

import dataclasses

import numpy as np

import concourse.bass as bass
import concourse.bacc as bacc
import concourse.tile as tile
from concourse import mybir
from concourse.bass_utils import run_bass_kernel_spmd
from concourse.masks import make_identity

F32 = mybir.dt.float32
U32 = mybir.dt.uint32

NCORES = 8
N, M, D = 16384, 8192, 3
ATOM_DIMS, CHEM, K = 6, 16, 16
NS = N // NCORES          # points per core
P = 128                   # points per chunk
NCHUNK = NS // P
MB = 512                  # atom block (psum bank free dim)
NMB = M // MB
TW = 20                   # gather-table row width (16 feat + 3 xyz + 1 pad)
NEG_BIG = -1.0e30


def _rep(ap, extra):
    """Append broadcast/custom [step, count] dims to an AP."""
    return dataclasses.replace(ap, ap=ap.ap + extra)


def _ap(ap, dims):
    """Replace the free dims of an AP (keep partition dim)."""
    return dataclasses.replace(ap, ap=[ap.ap[0]] + dims)


def build():
    nc = bacc.Bacc(None)

    xyz = nc.declare_dram_parameter("xyz", [NS, D], F32, isOutput=False)
    atom_xyz = nc.declare_dram_parameter("atom_xyz", [M, D], F32, isOutput=False)
    atomtypes = nc.declare_dram_parameter("atomtypes", [M, ATOM_DIMS], F32, isOutput=False)
    W_t1 = nc.declare_dram_parameter("W_t1", [CHEM, ATOM_DIMS], F32, isOutput=False)
    b_t1 = nc.declare_dram_parameter("b_t1", [CHEM, 1], F32, isOutput=False)
    W_t2 = nc.declare_dram_parameter("W_t2", [CHEM, CHEM], F32, isOutput=False)
    b_t2 = nc.declare_dram_parameter("b_t2", [CHEM, 1], F32, isOutput=False)
    W_t3 = nc.declare_dram_parameter("W_t3", [CHEM, CHEM], F32, isOutput=False)
    b_t3 = nc.declare_dram_parameter("b_t3", [CHEM, 1], F32, isOutput=False)
    W_att = nc.declare_dram_parameter("W_att", [1, K], F32, isOutput=False)
    W_e1 = nc.declare_dram_parameter("W_e1", [CHEM, CHEM], F32, isOutput=False)
    b_e1 = nc.declare_dram_parameter("b_e1", [CHEM, 1], F32, isOutput=False)
    W_e2 = nc.declare_dram_parameter("W_e2", [CHEM, CHEM], F32, isOutput=False)
    b_e2 = nc.declare_dram_parameter("b_e2", [CHEM, 1], F32, isOutput=False)
    W_e3 = nc.declare_dram_parameter("W_e3", [CHEM, CHEM], F32, isOutput=False)
    b_e3 = nc.declare_dram_parameter("b_e3", [CHEM, 1], F32, isOutput=False)
    out = nc.declare_dram_parameter("out", [NS, CHEM], F32, isOutput=True)
    dbg_ii = nc.declare_dram_parameter("dbg_ii", [NS, K], U32, isOutput=True)
    dbg_vv = nc.declare_dram_parameter("dbg_vv", [NS, K], F32, isOutput=True)
    dbg_fxn = nc.declare_dram_parameter("dbg_fxn", [NS, CHEM], F32, isOutput=True)
    dbg_u = nc.declare_dram_parameter("dbg_u", [NS, K], F32, isOutput=True)

    table = nc.dram_tensor("table", [M, TW], F32)

    ALU = mybir.AluOpType
    AX = mybir.AxisListType
    ACT_FN = mybir.ActivationFunctionType

    with tile.TileContext(nc) as tc:
        with (
            tc.tile_pool(name="const", bufs=1) as const,
            tc.tile_pool(name="psum_d", bufs=6, space="PSUM") as psum_d,
            tc.tile_pool(name="psum_s", bufs=2, space="PSUM") as psum_s,
        ):
            # ---------------- constants ----------------
            ident = const.tile([P, P], F32)
            make_identity(nc, ident[:, :])

            wt1 = const.tile([ATOM_DIMS, CHEM], F32)
            nc.sync.dma_start(out=wt1[:, :], in_=W_t1[:, :].rearrange("o i -> i o"))
            wt2 = const.tile([CHEM, CHEM], F32)
            nc.sync.dma_start(out=wt2[:, :], in_=W_t2[:, :].rearrange("o i -> i o"))
            wt3 = const.tile([CHEM, CHEM], F32)
            nc.sync.dma_start(out=wt3[:, :], in_=W_t3[:, :].rearrange("o i -> i o"))
            we1 = const.tile([CHEM, CHEM], F32)
            nc.sync.dma_start(out=we1[:, :], in_=W_e1[:, :].rearrange("o i -> i o"))
            we2 = const.tile([CHEM, CHEM], F32)
            nc.sync.dma_start(out=we2[:, :], in_=W_e2[:, :].rearrange("o i -> i o"))
            we3 = const.tile([CHEM, CHEM], F32)
            nc.sync.dma_start(out=we3[:, :], in_=W_e3[:, :].rearrange("o i -> i o"))
            bt1 = const.tile([CHEM, 1], F32)
            nc.sync.dma_start(out=bt1[:, :], in_=b_t1[:, :])
            bt2 = const.tile([CHEM, 1], F32)
            nc.sync.dma_start(out=bt2[:, :], in_=b_t2[:, :])
            bt3 = const.tile([CHEM, 1], F32)
            nc.sync.dma_start(out=bt3[:, :], in_=b_t3[:, :])
            be1 = const.tile([CHEM, 1], F32)
            nc.sync.dma_start(out=be1[:, :], in_=b_e1[:, :])
            be2 = const.tile([CHEM, 1], F32)
            nc.sync.dma_start(out=be2[:, :], in_=b_e2[:, :])
            be3 = const.tile([CHEM, 1], F32)
            nc.sync.dma_start(out=be3[:, :], in_=b_e3[:, :])

            # W_att broadcast to 128 partitions via ones-matmul
            watt = const.tile([1, K], F32)
            nc.sync.dma_start(out=watt[:, :], in_=W_att[:, :])
            ones1 = const.tile([1, P], F32)
            nc.vector.memset(ones1[:, :], 1.0)
            ps_w = psum_s.tile([P, K], F32, tag="ps_small")
            nc.tensor.matmul(ps_w[:, :], lhsT=ones1[:, :], rhs=watt[:, :],
                             start=True, stop=True)
            wattb = const.tile([P, K], F32)
            nc.scalar.copy(wattb[:, :], ps_w[:, :])

            # rhs4: rows 0-2 atom_xyz^T, row 3 |a|^2
            rhs4 = const.tile([4, M], F32)
            nc.sync.dma_start(out=rhs4[0:3, :], in_=atom_xyz[:, :].rearrange("m d -> d m"))

            # x2t: rows 0-2 2*xyz^T, row 3 -1  (lhsT for the distance matmul)
            x2t = const.tile([4, NS], F32)
            nc.vector.memset(x2t[:, :], -1.0)
            nc.sync.dma_start(out=x2t[0:3, :], in_=xyz[:, :].rearrange("n d -> d n"))
            nc.vector.tensor_scalar_mul(x2t[0:3, :], x2t[0:3, :], 2.0)

            with (
                tc.tile_pool(name="setup", bufs=1) as setup,
                tc.tile_pool(name="setup_t", bufs=2) as setup_t,
            ):
                # |a|^2 into rhs4 row 3
                asq = setup.tile([3, M], F32, tag="asq")
                nc.vector.tensor_mul(asq[:, :], rhs4[0:3, :], rhs4[0:3, :])
                ones3 = const.tile([3, 1], F32)
                nc.vector.memset(ones3[:, :], 1.0)
                anorm = setup.tile([1, M], F32, tag="anorm")
                for mb in range(NMB):
                    blk = slice(mb * MB, (mb + 1) * MB)
                    ps = psum_s.tile([1, MB], F32, tag="ps_small")
                    nc.tensor.matmul(ps[:, :], lhsT=ones3[:, :], rhs=asq[:, blk],
                                     start=True, stop=True)
                    nc.scalar.copy(anorm[0:1, blk], ps[:, :])
                nc.sync.dma_start(out=rhs4[3:4, :], in_=anorm[0:1, :])

                # atom feature MLP, [chem, atom] layout
                att = setup.tile([ATOM_DIMS, M], F32, tag="att")
                nc.sync.dma_start(out=att[:, :], in_=atomtypes[:, :].rearrange("m c -> c m"))

                def mlp_layer(dst, src, w, b, leaky=True):
                    for mb in range(NMB):
                        blk = slice(mb * MB, (mb + 1) * MB)
                        ps = psum_s.tile([CHEM, MB], F32, tag="ps_small")
                        nc.tensor.matmul(ps[:, :], lhsT=w[:, :], rhs=src[:, blk],
                                         start=True, stop=True)
                        nc.scalar.activation(dst[:, blk], ps[:, :], ACT_FN.Identity,
                                             bias=b[:, :])
                        if leaky:
                            nc.vector.scalar_tensor_tensor(
                                dst[:, blk], in0=dst[:, blk], scalar=0.2,
                                in1=dst[:, blk], op0=ALU.mult, op1=ALU.max)

                t1 = setup_t.tile([CHEM, M], F32, tag="tbuf")
                mlp_layer(t1, att, wt1, bt1)
                t2 = setup_t.tile([CHEM, M], F32, tag="tbuf")
                mlp_layer(t2, t1, wt2, bt2)
                t3 = setup_t.tile([CHEM, M], F32, tag="tbuf")
                mlp_layer(t3, t2, wt3, bt3)

                # gather table: [M, 20] = [t (16) | atom_xyz (3) | pad]
                nc.sync.dma_start(out=table[:, 0:CHEM].rearrange("m c -> c m"),
                                  in_=t3[:, :])
                nc.sync.dma_start(out=table[:, CHEM:CHEM + 3].rearrange("m d -> d m"),
                                  in_=rhs4[0:3, :])

            # ---------------- per-chunk loop ----------------
            with (
                tc.tile_pool(name="s_pool", bufs=2) as s_pool,
                tc.tile_pool(name="g_pool", bufs=2) as g_pool,
                tc.tile_pool(name="small", bufs=3) as small,
            ):
                for c in range(NCHUNK):
                    cpts = slice(c * P, (c + 1) * P)
                    # s = 2*x.a - |a|^2 (bigger == nearer)
                    s = s_pool.tile([P, M], F32, tag="s")
                    for mb in range(NMB):
                        blk = slice(mb * MB, (mb + 1) * MB)
                        ps = psum_d.tile([P, MB], F32, tag="ps_d")
                        nc.tensor.matmul(ps[:, :], lhsT=x2t[:, cpts],
                                         rhs=rhs4[:, blk], start=True, stop=True)
                        nc.scalar.copy(s[:, blk], ps[:, :])

                    # rank-ordered top-16 (values descending in s == nearest first)
                    vv = small.tile([P, K], F32, tag="vv")
                    ii = small.tile([P, K], U32, tag="ii")
                    nc.vector.max(out=vv[:, 0:8], in_=s[:, :])
                    nc.vector.max_index(out=ii[:, 0:8], in_max=vv[:, 0:8], in_values=s[:, :])
                    nc.vector.match_replace(out=s[:, :], in_to_replace=vv[:, 0:8],
                                            in_values=s[:, :], imm_value=NEG_BIG)
                    nc.vector.max(out=vv[:, 8:16], in_=s[:, :])
                    nc.vector.max_index(out=ii[:, 8:16], in_max=vv[:, 8:16], in_values=s[:, :])

                    # gather atom rows (feat 16 | xyz 3 | pad)
                    g = g_pool.tile([P, K, TW], F32, tag="g")
                    for k in range(K):
                        nc.gpsimd.indirect_dma_start(
                            out=g[:, k, :], out_offset=None,
                            in_=table[:, :],
                            in_offset=bass.IndirectOffsetOnAxis(ap=ii[:, k:k + 1], axis=0),
                        )

                    # per-point xyz
                    xs = small.tile([P, D], F32, tag="xs")
                    nc.sync.dma_start(out=xs[:, :], in_=xyz[cpts, :])

                    # vec_kd = x_d - a_kd (exact, like the reference)
                    vg = small.tile([P, K, D], F32, tag="vg")
                    nc.vector.tensor_tensor(
                        out=vg[:, :, :],
                        in0=_ap(xs[:, :], [[0, K], [1, D]]),
                        in1=g[:, :, CHEM:CHEM + 3],
                        op=ALU.subtract)
                    # dists_k = sum_d vec^2 ; w = 1/(dists + eps) ; u = w * W_att[k]
                    vsq = small.tile([P, K, D], F32, tag="vsq")
                    nc.vector.tensor_mul(vsq[:, :, :], vg[:, :, :], vg[:, :, :])
                    dd = small.tile([P, K], F32, tag="dd")
                    nc.vector.tensor_reduce(dd[:, :], vsq[:, :, :], axis=AX.X, op=ALU.add)
                    nc.vector.tensor_scalar_add(dd[:, :], dd[:, :], 1.0e-8)
                    w = small.tile([P, K], F32, tag="w")
                    nc.vector.reciprocal(w[:, :], dd[:, :])
                    u = small.tile([P, K], F32, tag="u")
                    nc.vector.tensor_mul(u[:, :], w[:, :], wattb[:, :])

                    # ut[k, c] = u_k * t_kc
                    ut = small.tile([P, K, CHEM], F32, tag="ut")
                    nc.vector.tensor_tensor(
                        out=ut[:, :, :],
                        in0=_rep(u[:, :], [[0, CHEM]]),
                        in1=g[:, :, 0:CHEM],
                        op=ALU.mult)
                    # tmp[c, d, k] = ut[k, c] * vec_kd
                    tmp = small.tile([P, CHEM, D, K], F32, tag="tmp")
                    nc.vector.tensor_tensor(
                        out=tmp[:, :, :, :],
                        in0=_ap(ut[:, :, :], [[1, CHEM], [0, D], [CHEM, K]]),
                        in1=_ap(vg[:, :, :], [[0, CHEM], [1, D], [D, K]]),
                        op=ALU.mult)
                    # F[c, d] = sum_k tmp[c, d, k]
                    Ft = small.tile([P, CHEM, D], F32, tag="Ft")
                    nc.vector.tensor_reduce(Ft[:, :, :], tmp[:, :, :, :],
                                            axis=AX.X, op=ALU.add)
                    # fx[c] = sqrt(sum_d F^2)
                    nc.vector.tensor_mul(Ft[:, :, :], Ft[:, :, :], Ft[:, :, :])
                    fx2 = small.tile([P, CHEM], F32, tag="fx2")
                    nc.vector.tensor_reduce(fx2[:, :], Ft[:, :, :], axis=AX.X, op=ALU.add)
                    fxn = small.tile([P, CHEM], F32, tag="fxn")
                    nc.scalar.sqrt(fxn[:, :], fx2[:, :])

                    # output MLP in [chem, point] layout
                    psT = psum_s.tile([CHEM, P], F32, tag="ps_small")
                    nc.tensor.transpose(psT[:, :], fxn[:, :], ident[:, :])
                    h0 = small.tile([CHEM, P], F32, tag="h0")
                    nc.scalar.copy(h0[:, :], psT[:, :])

                    ps1 = psum_s.tile([CHEM, P], F32, tag="ps_small")
                    nc.tensor.matmul(ps1[:, :], lhsT=we1[:, :], rhs=h0[:, :],
                                     start=True, stop=True)
                    h1 = small.tile([CHEM, P], F32, tag="h1")
                    nc.scalar.activation(h1[:, :], ps1[:, :], ACT_FN.Identity, bias=be1[:, :])
                    nc.vector.scalar_tensor_tensor(h1[:, :], in0=h1[:, :], scalar=0.2,
                                                   in1=h1[:, :], op0=ALU.mult, op1=ALU.max)

                    ps2 = psum_s.tile([CHEM, P], F32, tag="ps_small")
                    nc.tensor.matmul(ps2[:, :], lhsT=we2[:, :], rhs=h1[:, :],
                                     start=True, stop=True)
                    h2 = small.tile([CHEM, P], F32, tag="h2")
                    nc.scalar.activation(h2[:, :], ps2[:, :], ACT_FN.Identity, bias=be2[:, :])
                    nc.vector.scalar_tensor_tensor(h2[:, :], in0=h2[:, :], scalar=0.2,
                                                   in1=h2[:, :], op0=ALU.mult, op1=ALU.max)

                    ps3 = psum_s.tile([CHEM, P], F32, tag="ps_small")
                    nc.tensor.matmul(ps3[:, :], lhsT=we3[:, :], rhs=h2[:, :],
                                     start=True, stop=True)
                    h3 = small.tile([CHEM, P], F32, tag="h3")
                    nc.scalar.activation(h3[:, :], ps3[:, :], ACT_FN.Identity, bias=be3[:, :])

                    nc.sync.dma_start(out=out[cpts, :].rearrange("n c -> c n"),
                                      in_=h3[:, :])
                    nc.sync.dma_start(out=dbg_ii[cpts, :], in_=ii[:, :])
                    nc.sync.dma_start(out=dbg_vv[cpts, :], in_=vv[:, :])
                    nc.sync.dma_start(out=dbg_fxn[cpts, :], in_=fxn[:, :])
                    nc.sync.dma_start(out=dbg_u[cpts, :], in_=u[:, :])

    nc.compile()
    return nc


_NC = None


def _get_nc():
    global _NC
    if _NC is None:
        _NC = build()
    return _NC


def _in_maps(inputs):
    f = np.asarray
    shared = {
        "atom_xyz": f(inputs["atom_xyz"], dtype=np.float32),
        "atomtypes": f(inputs["atomtypes"], dtype=np.float32)[:, :ATOM_DIMS].copy(),
        "W_t1": f(inputs["W_t1"], dtype=np.float32),
        "b_t1": f(inputs["b_t1"], dtype=np.float32).reshape(CHEM, 1),
        "W_t2": f(inputs["W_t2"], dtype=np.float32),
        "b_t2": f(inputs["b_t2"], dtype=np.float32).reshape(CHEM, 1),
        "W_t3": f(inputs["W_t3"], dtype=np.float32),
        "b_t3": f(inputs["b_t3"], dtype=np.float32).reshape(CHEM, 1),
        "W_att": f(inputs["W_att"], dtype=np.float32).reshape(1, K),
        "W_e1": f(inputs["W_e1"], dtype=np.float32),
        "b_e1": f(inputs["b_e1"], dtype=np.float32).reshape(CHEM, 1),
        "W_e2": f(inputs["W_e2"], dtype=np.float32),
        "b_e2": f(inputs["b_e2"], dtype=np.float32).reshape(CHEM, 1),
        "W_e3": f(inputs["W_e3"], dtype=np.float32),
        "b_e3": f(inputs["b_e3"], dtype=np.float32).reshape(CHEM, 1),
    }
    xyz = f(inputs["xyz"], dtype=np.float32)
    return [
        {**shared, "xyz": xyz[i * NS:(i + 1) * NS].copy()}
        for i in range(NCORES)
    ]


def run(inputs, trace=False):
    nc = _get_nc()
    res = run_bass_kernel_spmd(nc, _in_maps(inputs), core_ids=list(range(NCORES)),
                               trace=trace)
    full = np.concatenate([res.results[i]["out"] for i in range(NCORES)], axis=0)
    return full, res


def kernel(**inputs):
    full, _ = run(inputs, trace=False)
    return full
